# revision 22
# baseline (speedup 1.0000x reference)
"""Trainium2 Bass kernel for a 7-layer ternary-weight (BitNet) 1D conv
feature extractor with exact-erf GELU after each layer.

Contract: kernel(**inputs) takes the FULL inputs from setup_inputs()
(x: [8, 160000] f32, w0..w6 / b0..b6 conv params) and returns the full
output [8, 256, 500] f32.

Strategy: data-parallel over batch - one batch element per NeuronCore.
Weights are ternarized on host (exact in fp8/fp16; the absmean scale
folds into the GELU's per-partition scale).

fp8 DoubleRow path: activations a0..a3 (inputs of L1..L4) are stored as
float8 e4m3; their convs run fp8 matmuls where pairs of 128-row k-panels
are fused into single DoubleRow instructions (2x contraction per column
streamed). Measured on HW: DR streams 1 column/cycle like fp16, so each
fused pair halves PE time for those taps. End-to-end quantization error
~0.7% rel-L2 (gate 2e-2). L0 input and a4/a5 stay fp16 (late-layer
quantization error is not attenuated enough).

Layout: per-layer mega-tile with phase regions [e | o | X | (Xdup)]:
 - e[t] = position 2t at col t+1, o[t] = position 2t+1 at col t+1
   (col 0 = zero halo), for channels 0-127.
 - X (cout>128 layers): channels 128-191 phase-STACKED: col c =
   [x[2c-2] on parts 0-63 ; x[2c-1] on parts 64-127].
 - Xdup = DMA copy of X so (X, X') DoubleRow pairs have non-overlapping
   access patterns (overlapping rhs panel APs hang the HW).
All tap reads are contiguous or stride-2 (both full rate on the PE).
DR panel pairs always cross regions (stride ~P), never overlap.
"""

import numpy as np
import ml_dtypes

E4NP = ml_dtypes.float8_e4m3

# (in_ch, out_ch, kernel, stride, pad) - fixed problem geometry
LAYERS = [(1, 128, 10, 5, 4), (128, 192, 3, 2, 1), (192, 192, 3, 2, 1),
          (192, 192, 3, 2, 1), (192, 256, 3, 2, 1), (256, 256, 4, 2, 1),
          (256, 256, 4, 2, 1)]
T_IN = 160000
LOUT = [32000, 16000, 8000, 4000, 2000, 1000, 500]
N_CORES = 8
NT = 512
A0C = 8192      # a0 chunk (position space) for the phase-A weave
A0H = A0C // 2  # ... in phase columns

# region pitches (cols) of the fp8 activation mega-tiles
PA = {0: 16002, 1: 8002, 2: 4002, 3: 2002}
# regions: e, o[, X[, Xdup]], ones (last region = all-1.0, the rhs for
# bias-injection DR panels)
NREG = {0: 3, 1: 5, 2: 5, 3: 4}

# fp8 weight blocks: name -> (ncols). Offsets assigned sequentially.
W8BLOCKS = [
    ("l1c0_p", 256), ("l1c0_s", 256),
    ("l1c1_a", 256), ("l1c1_b", 256), ("l1c1_c", 256),
    ("l2c0_p", 256), ("l2c0_q", 256), ("l2c0_r", 256),
    ("l2c1_a", 256), ("l2c1_b", 256), ("l2c1_c", 256), ("l2c1_d", 256),
    ("l3c0_p", 256), ("l3c0_q", 256), ("l3c0_r", 256),
    ("l3c1_a", 256), ("l3c1_b", 256), ("l3c1_c", 256), ("l3c1_d", 256),
    ("l4c0_p0", 256), ("l4c0_q0", 256), ("l4c0_r0", 256),
    ("l4c0_p1", 256), ("l4c0_q1", 256), ("l4c0_r1", 256),
]
W8COL = {}
_c = 0
for _n, _w in W8BLOCKS:
    W8COL[_n] = _c
    _c += _w
W8TOT = _c
# fp16 weights: L0 at col 0 (128), L5 at 128 (8*256), L6 at 2176 (8*256)
W16_L0, W16_L5, W16_L6 = 0, 128, 128 + 2048
W16TOT = 128 + 4096


def _bcol(i, mi):
    """Column of (bias, scale) pair for layer i, cout-chunk mi."""
    c = 0
    for j in range(i):
        c += 2 if LAYERS[j][1] <= 128 else 4
    return c + 2 * mi


def _signs(w):
    w = np.asarray(w, np.float32)
    scale = max(float(np.mean(np.abs(w))), 1e-5)
    sign = np.clip(np.round(w / scale), -1.0, 1.0)
    return sign, scale


def _pack_host(ws, bs):
    """Ternarize weights; pack fp16 block (L0/L5/L6), fp8 blocks
    (L1-L4 DR panel layouts), bias+scale (fp32)."""
    wpk16 = np.zeros((128, W16TOT), np.float16)
    wpk8 = np.zeros((128, W8TOT), np.float32)
    bpk = np.zeros((128, 26), np.float32)

    sgs = [_signs(w) for w in ws]

    # --- L0 (fp16): [11,128] block (row 10 = bias/scale), dup at 64 ---
    blk = np.zeros((11, 128), np.float16)
    blk[0:10] = sgs[0][0][:, 0, :].T.astype(np.float16)
    blk[10] = (np.asarray(bs[0], np.float32)[0:128]
               / sgs[0][1]).astype(np.float16)
    wpk16[0:11, W16_L0:W16_L0 + 128] = blk
    wpk16[64:75, W16_L0:W16_L0 + 128] = blk

    # --- L5/L6 (fp16): 8 groups (ti,kk) x [128, 256] ---
    for i, base in ((5, W16_L5), (6, W16_L6)):
        sign = sgs[i][0]
        g = 0
        for ti in range(2):
            for kk in range(4):
                wpk16[0:128, base + g * 256:base + (g + 1) * 256] = \
                    sign[:, 128 * ti:128 * ti + 128, kk].T
                g += 1

    def put(name, arr):
        c = W8COL[name]
        wpk8[:, c:c + arr.shape[1]] = arr

    def pan(*blocks):
        """Concatenate 128x128 panels horizontally."""
        return np.concatenate(blocks, axis=1)

    def z():
        return np.zeros((128, 128), np.float32)

    def quad(tl, tr, bl, br):
        """Build a 128x128 from 64x64-ish quadrant blocks (None=0)."""
        m = np.zeros((128, 128), np.float32)
        if tl is not None:
            m[0:64, 0:64] = tl
        if tr is not None:
            m[0:64, 64:128] = tr
        if bl is not None:
            m[64:128, 0:64] = bl
        if br is not None:
            m[64:128, 64:128] = br
        return m

    # --- L1 (cin 128, cout 192) ---
    s1 = sgs[1][0]
    W = [s1[0:128, :, k].T for k in range(3)]       # [128cin, 128cout]
    Wc = [s1[128:192, 0:128, k].T for k in range(3)]  # [128cin, 64cout]
    def brow(vals):
        m = np.zeros((128, 128), np.float32)
        m[0, 0:len(vals)] = vals
        return m

    b1 = np.asarray(bs[1], np.float32) / sgs[1][1]
    put("l1c0_p", pan(W[1], W[0]))
    put("l1c0_s", pan(W[2], brow(b1[0:128])))

    def c1w(left, right):
        m = np.zeros((128, 128), np.float32)
        if left is not None:
            m[:, 0:64] = left
        if right is not None:
            m[:, 64:128] = right
        return m

    put("l1c1_a", pan(c1w(Wc[1], None), c1w(Wc[0], None)))
    put("l1c1_b", pan(c1w(None, Wc[1]), c1w(Wc[2], Wc[0])))
    put("l1c1_c", pan(c1w(None, Wc[2]),
                      brow(np.concatenate([b1[128:192], b1[128:192]]))))

    # --- L2/L3 (cin 192, cout 192), L4 (cin 192, cout 256) ---
    for i in (2, 3, 4):
        sign = sgs[i][0]
        cout = LAYERS[i][1]
        nchunk = 1 if cout == 192 else 2
        for mi in range(nchunk):
            co = slice(128 * mi, 128 * mi + 128)
            Wf = [sign[co, 0:128, k].T for k in range(3)]
            Wx = [sign[co, 128:192, k].T for k in range(3)]  # [64, 128]
            sfx = str(mi) if i == 4 else ""
            comb = np.zeros((128, 128), np.float32)
            comb[0:64, :] = Wx[1]
            comb[64:128, :] = Wx[2]
            r = np.zeros((128, 128), np.float32)
            r[64:128, :] = Wx[0]
            bi = np.asarray(bs[i], np.float32) / sgs[i][1]
            put(f"l{i}c0_p{sfx}", pan(Wf[1], Wf[0]))
            put(f"l{i}c0_q{sfx}", pan(Wf[2], comb))
            put(f"l{i}c0_r{sfx}", pan(r, brow(bi[128 * mi:128 * mi + 128])))
        if cout == 192:
            Wc_ = [sign[128:192, 0:128, k].T for k in range(3)]  # [128,64]
            Wxc = [sign[128:192, 128:192, k].T for k in range(3)]  # [64,64]
            put(f"l{i}c1_a", pan(c1w(Wc_[1], None), c1w(Wc_[0], None)))
            put(f"l{i}c1_b", pan(c1w(None, Wc_[1]), c1w(Wc_[2], Wc_[0])))
            xc1 = quad(Wxc[1], None, Wxc[2], Wxc[0])
            put(f"l{i}c1_c", pan(c1w(None, Wc_[2]), xc1))
            xd0 = quad(None, None, Wxc[0], None)
            xd1 = quad(None, Wxc[1], None, Wxc[2])
            put(f"l{i}c1_d", pan(xd0, xd1))

    # --- bias + scale ---
    bcol = 0
    for i, (cin, cout, k, s, p) in enumerate(LAYERS):
        scale = sgs[i][1]
        b = np.asarray(bs[i], np.float32)
        bpk[0:128, bcol] = b[0:128]
        bpk[0:128, bcol + 1] = scale
        bcol += 2
        if cout > 128:
            if cout == 192:   # stacked for phase-stacked psum
                bpk[0:64, bcol] = b[128:192]
                bpk[64:128, bcol] = b[128:192]
            else:
                bpk[0:128, bcol] = b[128:256]
            bpk[0:128, bcol + 1] = scale
            bcol += 2
    return wpk16, wpk8.astype(E4NP), bpk


def _prep_x(xb):
    """Per-core L0 input, phase-reordered: cols [0:16000) hold the
    window for even L0 outputs, [16000:) for odd. xr[j, col] =
    xpad[5t + j] with t = 2*col (col<16000) or 2*(col-16000)+1."""
    xpad = np.zeros(T_IN + 16, np.float16)
    xpad[4:4 + T_IN] = xb.astype(np.float16)
    L = LOUT[0]
    xr = np.empty((11, L), np.float16)
    for j in range(10):
        xr[j, :] = xpad[j:j + 5 * L:5]
    xr[10, :] = 1.0    # bias-injection row
    return np.concatenate([xr[:, 0::2], xr[:, 1::2]], axis=1)


_CACHE = {}


def _build(act_name="Gelu", scales=(1.0,) * 7):
    """Build + compile the Bass program. The 7 ternary scales are baked
    in as ACT immediates (explicit scale/bias APs cost ~270ns per act)."""
    key = ("nc", act_name) + tuple(float(np.float32(s)) for s in scales)
    if key in _CACHE:
        return _CACHE[key]
    from concourse import bacc
    import concourse.mybir as mybir
    import concourse.tile as tile
    import concourse.bass as bass

    F8 = mybir.dt.float8e4
    F16 = mybir.dt.float16
    F32 = mybir.dt.float32
    ACTF = getattr(mybir.ActivationFunctionType, act_name)
    DRM = mybir.MatmulPerfMode.DoubleRow

    nc = bacc.Bacc("TRN2")
    xr_d = nc.dram_tensor("xr", [11, LOUT[0]], F16, kind="ExternalInput")
    on_d = nc.dram_tensor("on8", [128, PA[0]], F8, kind="ExternalInput")
    w16_d = nc.dram_tensor("w16", [128, W16TOT], F16, kind="ExternalInput")
    w8_d = nc.dram_tensor("w8", [128, W8TOT], F8, kind="ExternalInput")
    bp_d = nc.dram_tensor("bp", [128, 26], F32, kind="ExternalInput")
    y_d = nc.dram_tensor("y", [256, 500], F32, kind="ExternalOutput")

    with tile.TileContext(nc) as tc:
        pools = []

        def mkpool(name, bufs=1, space="SBUF"):
            p = tc.alloc_tile_pool(name=name, bufs=bufs, space=space)
            pools.append(p)
            return p

        wpool = mkpool("wpool")
        wt16 = wpool.tile([128, W16TOT], F16, name="wt16")
        wt8 = wpool.tile([128, W8TOT], F8, name="wt8")
        bt = wpool.tile([128, 26], F32, name="bt")

        opool = mkpool("opool")
        stage = opool.tile([128, 1000], F32, name="stage")
        scratch = opool.tile([128, 512], F16, name="scratch")
        xpool = mkpool("xpool", bufs=3)


        lpool = mkpool("lpool")
        A0 = lpool.tile([128, NREG[0], PA[0]], F8, name="A0")
        A1 = lpool.tile([128, NREG[1], PA[1]], F8, name="A1")
        A2 = lpool.tile([128, NREG[2], PA[2]], F8, name="A2")
        A3 = lpool.tile([128, NREG[3], PA[3]], F8, name="A3")
        A4a = lpool.tile([128, 2, 1004], F16, name="A4a")
        A4b = lpool.tile([128, 2, 1004], F16, name="A4b")
        A5a = lpool.tile([128, 2, 504], F16, name="A5a")
        A5b = lpool.tile([128, 2, 504], F16, name="A5b")
        AT = {0: A0, 1: A1, 2: A2, 3: A3}

        # zero halos (col 0 of every region; e-tail halo for L5/L6 srcs)
        for i in (0, 1, 2, 3):
            t, P, nr = AT[i], PA[i], NREG[i]
            for r in range(nr):
                nc.vector.memset(
                    bass.AP(t.tensor, r * P, [[nr * P, 128], [1, 1]]), 0.0)
        for i in (0, 1, 2, 3):    # ones region = last region index
            t, P, nr = AT[i], PA[i], NREG[i]
            nc.sync.dma_start(
                out=bass.AP(t.tensor, (nr - 1) * P, [[nr * P, 128], [1, P]]),
                in_=on_d.ap()[:, 0:P])
        for t in (A4a, A4b):
            nc.vector.memset(t[:, 1, 0:1], 0.0)       # o halo
            nc.vector.memset(t[:, 0, 1001:1002], 0.0)  # e[1000]
        for t in (A5a, A5b):
            nc.vector.memset(t[:, 1, 0:1], 0.0)
            nc.vector.memset(t[:, 0, 501:502], 0.0)

        # PSUM: one deep rotating pool - 8 units in flight in phase A
        # ([128,1024] x 8 banks), 4 in phase B ([128,2048] x 4).
        poolPS = tc.alloc_tile_pool(name="poolPS", bufs=4, space="PSUM")
        cur = {"pool": poolPS, "w": 1024}

        def ps_tile():
            return cur["pool"].tile([128, cur["w"]], F32, name="pps",
                                    tag="ps")

        fa_tile = ps_tile
        fb_tile = ps_tile

        nc.vector.memset(scratch[:, :], 0.0)

        def junk_mms(n):
            jp = fb_tile()
            for _ in range(n):
                nc.tensor.matmul(jp[:, 0:512], scratch[:, 0:128],
                                 scratch[:, :], start=True, stop=True)

        junk_mms(14)

        def w8ap(name, panels):
            c = W8COL[name]
            if panels == 2:
                return bass.AP(wt8.tensor, c, [[W8TOT, 128], [128, 2],
                                               [1, 128]])
            return bass.AP(wt8.tensor, c, [[W8TOT, 128], [1, 128]])

        def act_eo(dst, i, mi, u0, n, ps, pitchcols, P, with_bias=True):
            """One-call phase-scatter GELU: psum [128, n] -> e/o regions
            (fp16 dst tiles: direct from ACT)."""
            c = _bcol(i, mi)
            kw = dict(bias=bt[0:128, c:c + 1])
            nc.scalar.activation(
                bass.AP(dst.tensor, 1 + u0 // 2,
                        [[pitchcols, 128], [P, 2], [1, n // 2]]),
                bass.AP(ps.tensor, ps.offset,
                        [[ps.ap[0][0], 128], [1, 2], [2, n // 2]]),
                ACTF, scale=bt[0:128, c + 1:c + 2], **kw)

        def act_eo8(dst, i, mi, u0, n, ps, pitchcols, P):
            """Phase-scatter GELU, fp8 dst, bias pre-injected via matmul
            ones-panels, scale as immediate: minimal ACT overhead."""
            nc.scalar.activation(
                bass.AP(dst.tensor, 1 + u0 // 2,
                        [[pitchcols, 128], [P, 2], [1, n // 2]]),
                bass.AP(ps.tensor, ps.offset,
                        [[ps.ap[0][0], 128], [1, 2], [2, n // 2]]),
                ACTF, bias=bt[0:128, 0:1], scale=bt[0:128, 1:2])

        def act_c18(dstap, ps, h, i, mi, with_bias):
            """c1 (stacked) GELU -> X region. deep-c1 has no free DR slot
            for bias injection, so those acts keep an explicit bias AP."""
            c = _bcol(i, mi)
            nc.scalar.activation(dstap, ps[0:128, 0:h], ACTF,
                                 bias=bt[0:128, c:c + 1],
                                 scale=bt[0:128, c + 1:c + 2])

        def act_plain(dstap, srcap, i, mi, with_bias=True):
            c = _bcol(i, mi)
            nc.scalar.activation(dstap, srcap, ACTF,
                                 bias=bt[0:128, c:c + 1],
                                 scale=bt[0:128, c + 1:c + 2])

        # ---------------- L1 units ----------------
        def l1_c0_unit(u0, n):
            ps = fa_tile()
            P = PA[0]
            for s0 in range(0, n, NT):
                w = min(NT, n - s0)
                u = u0 + s0
                nc.tensor.matmul(
                    ps[:, s0:s0 + w], w8ap("l1c0_p", 2),
                    bass.AP(A0.tensor, u + 1,
                            [[3 * P, 128], [P - 1, 2], [1, w]]),
                    start=True, stop=False, perf_mode=DRM)
                nc.tensor.matmul(
                    ps[:, s0:s0 + w], w8ap("l1c0_s", 2),
                    bass.AP(A0.tensor, P + u + 1,
                            [[3 * P, 128], [P, 2], [1, w]]),
                    start=False, stop=True, perf_mode=DRM)
            act_eo8(A1, 1, 0, u0, n, ps, 5 * PA[1], PA[1])

        def l1_c1_unit(m0, h):
            ps = fb_tile()
            P = PA[0]
            for b0 in range(0, h, NT):
                w = min(NT, h - b0)
                m = m0 + b0
                nc.tensor.matmul(
                    ps[:, b0:b0 + w], w8ap("l1c1_a", 2),
                    bass.AP(A0.tensor, 2 * m + 1,
                            [[3 * P, 128], [P - 1, 2], [2, w]]),
                    start=True, stop=False, perf_mode=DRM)
                nc.tensor.matmul(
                    ps[:, b0:b0 + w], w8ap("l1c1_b", 2),
                    bass.AP(A0.tensor, 2 * m + 2,
                            [[3 * P, 128], [P - 1, 2], [2, w]]),
                    start=False, stop=False, perf_mode=DRM)
                nc.tensor.matmul(
                    ps[:, b0:b0 + w], w8ap("l1c1_c", 2),
                    bass.AP(A0.tensor, P + 2 * m + 2,
                            [[3 * P, 128], [P - 1, 2], [2, w]]),
                    start=False, stop=True, perf_mode=DRM)
            P1 = PA[1]
            act_c18(bass.AP(A1.tensor, 2 * P1 + m0 + 1,
                            [[5 * P1, 128], [1, h]]), ps, h, 1, 1, False)
            nc.sync.dma_start(
                out=bass.AP(A1.tensor, 3 * P1 + m0 + 1,
                            [[5 * P1, 128], [1, h]]),
                in_=bass.AP(A1.tensor, 2 * P1 + m0 + 1,
                            [[5 * P1, 128], [1, h]]))

        # ---------------- deep units (L2-L4) ----------------
        def deep_c0_unit(i, u0, n, mi):
            src, P = AT[i - 1], PA[i - 1]
            nr = NREG[i - 1]
            sfx = str(mi) if i == 4 else ""
            ps = fa_tile()
            for s0 in range(0, n, NT):
                w = min(NT, n - s0)
                u = u0 + s0
                nc.tensor.matmul(
                    ps[:, s0:s0 + w], w8ap(f"l{i}c0_p{sfx}", 2),
                    bass.AP(src.tensor, u + 1,
                            [[nr * P, 128], [P - 1, 2], [1, w]]),
                    start=True, stop=False, perf_mode=DRM)
                nc.tensor.matmul(
                    ps[:, s0:s0 + w], w8ap(f"l{i}c0_q{sfx}", 2),
                    bass.AP(src.tensor, P + u + 1,
                            [[nr * P, 128], [P, 2], [1, w]]),
                    start=False, stop=False, perf_mode=DRM)
                nc.tensor.matmul(
                    ps[:, s0:s0 + w], w8ap(f"l{i}c0_r{sfx}", 2),
                    bass.AP(src.tensor, 2 * P + u,
                            [[nr * P, 128], [(nr - 3) * P, 2], [1, w]]),
                    start=False, stop=True, perf_mode=DRM)
            if i < 4:
                act_eo8(AT[i], i, 0, u0, n, ps, NREG[i] * PA[i], PA[i])
            else:
                dst = A4a if mi == 0 else A4b
                act_eo(dst, 4, mi, u0, n, ps, 2 * 1004, 1004, False)

        def deep_c1_unit(i, m0, h):
            src, P = AT[i - 1], PA[i - 1]
            nr = NREG[i - 1]
            ps = fb_tile()
            for b0 in range(0, h, NT):
                w = min(NT, h - b0)
                m = m0 + b0
                nc.tensor.matmul(
                    ps[:, b0:b0 + w], w8ap(f"l{i}c1_a", 2),
                    bass.AP(src.tensor, 2 * m + 1,
                            [[nr * P, 128], [P - 1, 2], [2, w]]),
                    start=True, stop=False, perf_mode=DRM)
                nc.tensor.matmul(
                    ps[:, b0:b0 + w], w8ap(f"l{i}c1_b", 2),
                    bass.AP(src.tensor, 2 * m + 2,
                            [[nr * P, 128], [P - 1, 2], [2, w]]),
                    start=False, stop=False, perf_mode=DRM)
                nc.tensor.matmul(
                    ps[:, b0:b0 + w], w8ap(f"l{i}c1_c", 2),
                    bass.AP(src.tensor, P + 2 * m + 2,
                            [[nr * P, 128], [P - 1, 2], [2, w]]),
                    start=False, stop=False, perf_mode=DRM)
                nc.tensor.matmul(
                    ps[:, b0:b0 + w], w8ap(f"l{i}c1_d", 2),
                    bass.AP(src.tensor, 2 * P + 2 * m,
                            [[nr * P, 128], [P + 2, 2], [2, w]]),
                    start=False, stop=True, perf_mode=DRM)
            Pi = PA[i]
            nri = NREG[i]
            act_c18(bass.AP(AT[i].tensor, 2 * Pi + m0 + 1,
                            [[nri * Pi, 128], [1, h]]), ps, h, i, 1, True)
            if nri == 5:    # tiles with an Xdup region (A1, A2)
                nc.sync.dma_start(
                    out=bass.AP(AT[i].tensor, 3 * Pi + m0 + 1,
                                [[nri * Pi, 128], [1, h]]),
                    in_=bass.AP(AT[i].tensor, 2 * Pi + m0 + 1,
                                [[nri * Pi, 128], [1, h]]))

        # ---------------- L5/L6 std units (fp16) ----------------
        def std_unit(i, mi):
            (ta, tb) = (A4a, A4b) if i == 5 else (A5a, A5b)
            Pp = 1004 if i == 5 else 504
            base = W16_L5 if i == 5 else W16_L6
            lout = LOUT[i]
            ps = ps_tile()
            g = 0
            for ti in range(2):
                t = ta if ti == 0 else tb
                # taps: o[v-1] (o col v), e[v] (e col v+1),
                #       o[v] (o col v+1), e[v+1] (e col v+2)
                srcs = [(Pp, 0), (0, 1), (Pp, 1), (0, 2)]
                for kk in range(4):
                    lhsT = wt16[0:128, base + g * 256 + 128 * mi:
                                base + g * 256 + 128 * mi + 128]
                    roff, cofs = srcs[kk]
                    for s0 in range(0, lout, NT):
                        w = min(NT, lout - s0)
                        nc.tensor.matmul(
                            ps[:, s0:s0 + w], lhsT,
                            bass.AP(t.tensor, roff + s0 + cofs,
                                    [[2 * Pp, 128], [1, w]]),
                            start=(g == 0), stop=(g == 7))
                    g += 1
            if i == 5:
                dst = A5a if mi == 0 else A5b
                act_eo(dst, 5, mi, 0, lout, ps, 2 * 504, 504)
            else:
                act_plain(stage[0:128, 500 * mi:500 * mi + lout],
                          ps[0:128, 0:lout], 6, mi)

        # ============ phase A: L0 streamed; L1 + early L2 woven ========
        wrest = [0]

        def after_first_xt():
            if wrest[0] == 1:
                nc.gpsimd.dma_start(out=wt8[:, :], in_=w8_d.ap())
                nc.gpsimd.dma_start(out=wt16[:, 128:W16TOT],
                                    in_=w16_d.ap()[:, 128:W16TOT])
            wrest[0] += 1

        n_ch = (LOUT[0] + A0C - 1) // A0C
        for c in range(n_ch):
            cb2 = c * A0H
            csz2 = min(A0H, LOUT[0] // 2 - cb2)
            fillers = []
            if c > 0:
                pb = (c - 1) * A0C // 2     # L1 position base of chunk c-1
                for j in range(4):
                    fillers.append(lambda u0=pb + 1024 * j:
                                   l1_c0_unit(u0, 1024))
                    if j % 2 == 1:
                        fillers.append(lambda m0=pb // 2 + 512 * (j - 1):
                                       l1_c1_unit(m0, 1024))
            if c >= 2:
                for j in range(2):
                    fillers.append(lambda u0=(c - 2) * 2048 + 1024 * j:
                                   deep_c0_unit(2, u0, 1024, 0))
                fillers.append(lambda m0=(c - 2) * 1024:
                               deep_c1_unit(2, m0, 1024))
            if c == 0:
                fillers = [lambda: junk_mms(2) for _ in range(4)]
            fi = 0
            nst = 2 * ((csz2 + 1023) // 1024)
            for sti in range(nst):
                ph = sti % 2
                t0 = cb2 + 1024 * (sti // 2)
                stw = min(1024, cb2 + csz2 - t0)
                xt = xpool.tile([128, NT], F16, tag="xt",
                                name=f"xt{ph}_{t0}")
                if t0 == 0 and ph == 0:
                    nc.sync.dma_start(out=wt16[:, 0:128],
                                      in_=w16_d.ap()[:, 0:128])
                    nc.sync.dma_start(out=bt[:, :], in_=bp_d.ap())
                for s in range(0, stw, NT):
                    w = min(NT, stw - s)
                    g = (s // NT) * 64
                    nc.sync.dma_start(
                        out=xt[g:g + 11, 0:w],
                        in_=xr_d.ap()[:, 16000 * ph + t0 + s:
                                      16000 * ph + t0 + s + w])
                after_first_xt()
                ps = ps_tile()
                for s in range(0, stw, NT):
                    w = min(NT, stw - s)
                    g = (s // NT) * 64
                    nc.tensor.matmul(ps[:, s:s + w],
                                     wt16[g:g + 10, 0:128],
                                     xt[g:g + 10, 0:w],
                                     start=True, stop=True,
                                     tile_position=(g, 0))
                act_plain(bass.AP(A0.tensor, ph * PA[0] + t0 + 1,
                                  [[3 * PA[0], 128], [1, stw]]),
                          ps[0:128, 0:stw], 0, 0, True)
                if sti % 2 == 1 and fi < len(fillers):
                    fillers[fi]()
                    fi += 1
            while fi < len(fillers):
                fillers[fi]()
                fi += 1
        # drain: L1 of last chunk (clamped to LOUT[1])
        pb = (n_ch - 1) * A0C // 2
        for j in range(4):
            u0 = pb + 1024 * j
            if u0 < LOUT[1]:
                l1_c0_unit(u0, min(1024, LOUT[1] - u0))
            if j % 2 == 1:
                m0 = pb // 2 + 512 * (j - 1)
                if m0 < LOUT[1] // 2:
                    l1_c1_unit(m0, min(1024, LOUT[1] // 2 - m0))

        # ============ phase B: rest of L2, then L3..L6 =================

        deep_c0_unit(2, 4096, 1024, 0)
        deep_c1_unit(2, 2048, 1024)
        deep_c0_unit(2, 5120, 1024, 0)
        deep_c1_unit(2, 3072, 928)
        deep_c0_unit(2, 6144, 1024, 0)
        deep_c0_unit(2, 7168, 832, 0)
        deep_c0_unit(3, 0, 1024, 0)
        deep_c1_unit(3, 0, 1024)
        deep_c0_unit(3, 1024, 1024, 0)
        deep_c1_unit(3, 1024, 976)
        deep_c0_unit(3, 2048, 1024, 0)
        deep_c0_unit(3, 3072, 928, 0)
        deep_c0_unit(4, 0, 1024, 0)
        deep_c0_unit(4, 0, 1024, 1)
        deep_c0_unit(4, 1024, 976, 0)
        deep_c0_unit(4, 1024, 976, 1)
        std_unit(5, 0)
        std_unit(5, 1)
        std_unit(6, 0)
        std_unit(6, 1)

        nc.sync.dma_start(out=y_d.ap()[0:128, :], in_=stage[:, 0:500])
        nc.sync.dma_start(out=y_d.ap()[128:256, :], in_=stage[:, 500:1000])
        poolPS.release()
        for p in reversed(pools):
            p.release()

    nc.compile()
    _CACHE[key] = nc
    return nc


def kernel(x, w0, b0, w1, b1, w2, b2, w3, b3, w4, b4, w5, b5, w6, b6):
    import os
    from concourse.bass_utils import run_bass_kernel_spmd

    ws = [w0, w1, w2, w3, w4, w5, w6]
    bs = [b0, b1, b2, b3, b4, b5, b6]
    wpk16, wpk8, bpk = _pack_host(ws, bs)
    scales = tuple(_signs(w)[1] for w in ws)
    ones8 = np.ones((128, PA[0]), E4NP)
    x = np.asarray(x, np.float32)
    in_maps = [{"xr": _prep_x(x[b]), "w16": wpk16, "w8": wpk8, "bp": bpk,
                "on8": ones8}
               for b in range(N_CORES)]
    nc = _build("Gelu", scales)
    trace = bool(os.environ.get("BITCONV_TRACE"))
    res = run_bass_kernel_spmd(nc, in_maps, core_ids=list(range(N_CORES)),
                               trace=trace)
    if trace:
        print(f"HW exec time: {res.exec_time_ns} ns")
        _CACHE["last_results"] = res
    return np.stack([res.results[b]["y"] for b in range(N_CORES)], axis=0)


# revision 23
# speedup vs baseline: 1.0158x; 1.0158x over previous
"""Trainium2 Bass kernel for a 7-layer ternary-weight (BitNet) 1D conv
feature extractor with exact-erf GELU after each layer.

Contract: kernel(**inputs) takes the FULL inputs from setup_inputs()
(x: [8, 160000] f32, w0..w6 / b0..b6 conv params) and returns the full
output [8, 256, 500] f32.

Strategy: data-parallel over batch - one batch element per NeuronCore.
Weights are ternarized on host (exact in fp8/fp16; the absmean scale
folds into the GELU's per-partition scale).

fp8 DoubleRow path: activations a0..a3 (inputs of L1..L4) are stored as
float8 e4m3; their convs run fp8 matmuls where pairs of 128-row k-panels
are fused into single DoubleRow instructions (2x contraction per column
streamed). Measured on HW: DR streams 1 column/cycle like fp16, so each
fused pair halves PE time for those taps. End-to-end quantization error
~0.7% rel-L2 (gate 2e-2). L0 input and a4/a5 stay fp16 (late-layer
quantization error is not attenuated enough).

Layout: per-layer mega-tile with phase regions [e | o | X | (Xdup)]:
 - e[t] = position 2t at col t+1, o[t] = position 2t+1 at col t+1
   (col 0 = zero halo), for channels 0-127.
 - X (cout>128 layers): channels 128-191 phase-STACKED: col c =
   [x[2c-2] on parts 0-63 ; x[2c-1] on parts 64-127].
 - Xdup = DMA copy of X so (X, X') DoubleRow pairs have non-overlapping
   access patterns (overlapping rhs panel APs hang the HW).
All tap reads are contiguous or stride-2 (both full rate on the PE).
DR panel pairs always cross regions (stride ~P), never overlap.
"""

import numpy as np
import ml_dtypes

E4NP = ml_dtypes.float8_e4m3

# (in_ch, out_ch, kernel, stride, pad) - fixed problem geometry
LAYERS = [(1, 128, 10, 5, 4), (128, 192, 3, 2, 1), (192, 192, 3, 2, 1),
          (192, 192, 3, 2, 1), (192, 256, 3, 2, 1), (256, 256, 4, 2, 1),
          (256, 256, 4, 2, 1)]
T_IN = 160000
LOUT = [32000, 16000, 8000, 4000, 2000, 1000, 500]
N_CORES = 8
NT = 512
A0C = 8192      # a0 chunk (position space) for the phase-A weave
A0H = A0C // 2  # ... in phase columns

# region pitches (cols) of the fp8 activation mega-tiles
PA = {0: 16002, 1: 8002, 2: 4002, 3: 2002}
# regions: e, o[, X[, Xdup]], ones (last region = all-1.0, the rhs for
# bias-injection DR panels)
NREG = {0: 3, 1: 5, 2: 5, 3: 4}

# fp8 weight blocks: name -> (ncols). Offsets assigned sequentially.
W8BLOCKS = [
    ("l1c0_p", 256), ("l1c0_s", 256),
    ("l1c1_a", 256), ("l1c1_b", 256), ("l1c1_c", 256),
    ("l2c0_p", 256), ("l2c0_q", 256), ("l2c0_r", 256),
    ("l2c1_a", 256), ("l2c1_b", 256), ("l2c1_c", 256), ("l2c1_d", 256),
    ("l3c0_p", 256), ("l3c0_q", 256), ("l3c0_r", 256),
    ("l3c1_a", 256), ("l3c1_b", 256), ("l3c1_c", 256), ("l3c1_d", 256),
    ("l4c0_p0", 256), ("l4c0_q0", 256), ("l4c0_r0", 256),
    ("l4c0_p1", 256), ("l4c0_q1", 256), ("l4c0_r1", 256),
]
W8COL = {}
_c = 0
for _n, _w in W8BLOCKS:
    W8COL[_n] = _c
    _c += _w
W8TOT = _c
# fp16 weights: L0 at col 0 (128), L5 at 128 (8*256), L6 at 2176 (8*256)
W16_L0, W16_L5, W16_L6 = 0, 128, 128 + 2048
W16TOT = 128 + 4096


def _bcol(i, mi):
    """Column of (bias, scale) pair for layer i, cout-chunk mi."""
    c = 0
    for j in range(i):
        c += 2 if LAYERS[j][1] <= 128 else 4
    return c + 2 * mi


def _signs(w):
    w = np.asarray(w, np.float32)
    scale = max(float(np.mean(np.abs(w))), 1e-5)
    sign = np.clip(np.round(w / scale), -1.0, 1.0)
    return sign, scale


def _pack_host(ws, bs):
    """Ternarize weights; pack fp16 block (L0/L5/L6), fp8 blocks
    (L1-L4 DR panel layouts), bias+scale (fp32)."""
    wpk16 = np.zeros((128, W16TOT), np.float16)
    wpk8 = np.zeros((128, W8TOT), np.float32)
    bpk = np.zeros((128, 26), np.float32)

    sgs = [_signs(w) for w in ws]

    # --- L0 (fp16): [11,128] block (row 10 = bias/scale), dup at 64 ---
    blk = np.zeros((11, 128), np.float16)
    blk[0:10] = sgs[0][0][:, 0, :].T.astype(np.float16)
    blk[10] = (np.asarray(bs[0], np.float32)[0:128]
               / sgs[0][1]).astype(np.float16)
    wpk16[0:11, W16_L0:W16_L0 + 128] = blk
    wpk16[64:75, W16_L0:W16_L0 + 128] = blk

    # --- L5/L6 (fp16): 8 groups (ti,kk) x [128, 256] ---
    for i, base in ((5, W16_L5), (6, W16_L6)):
        sign = sgs[i][0]
        g = 0
        for ti in range(2):
            for kk in range(4):
                wpk16[0:128, base + g * 256:base + (g + 1) * 256] = \
                    sign[:, 128 * ti:128 * ti + 128, kk].T
                g += 1

    def put(name, arr):
        c = W8COL[name]
        wpk8[:, c:c + arr.shape[1]] = arr

    def pan(*blocks):
        """Concatenate 128x128 panels horizontally."""
        return np.concatenate(blocks, axis=1)

    def z():
        return np.zeros((128, 128), np.float32)

    def quad(tl, tr, bl, br):
        """Build a 128x128 from 64x64-ish quadrant blocks (None=0)."""
        m = np.zeros((128, 128), np.float32)
        if tl is not None:
            m[0:64, 0:64] = tl
        if tr is not None:
            m[0:64, 64:128] = tr
        if bl is not None:
            m[64:128, 0:64] = bl
        if br is not None:
            m[64:128, 64:128] = br
        return m

    # --- L1 (cin 128, cout 192) ---
    s1 = sgs[1][0]
    W = [s1[0:128, :, k].T for k in range(3)]       # [128cin, 128cout]
    Wc = [s1[128:192, 0:128, k].T for k in range(3)]  # [128cin, 64cout]
    def brow(vals):
        m = np.zeros((128, 128), np.float32)
        m[0, 0:len(vals)] = vals
        return m

    b1 = np.asarray(bs[1], np.float32) / sgs[1][1]
    put("l1c0_p", pan(W[1], W[0]))
    put("l1c0_s", pan(W[2], brow(b1[0:128])))

    def c1w(left, right):
        m = np.zeros((128, 128), np.float32)
        if left is not None:
            m[:, 0:64] = left
        if right is not None:
            m[:, 64:128] = right
        return m

    put("l1c1_a", pan(c1w(Wc[1], None), c1w(Wc[0], None)))
    put("l1c1_b", pan(c1w(None, Wc[1]), c1w(Wc[2], Wc[0])))
    put("l1c1_c", pan(c1w(None, Wc[2]),
                      brow(np.concatenate([b1[128:192], b1[128:192]]))))

    # --- L2/L3 (cin 192, cout 192), L4 (cin 192, cout 256) ---
    for i in (2, 3, 4):
        sign = sgs[i][0]
        cout = LAYERS[i][1]
        nchunk = 1 if cout == 192 else 2
        for mi in range(nchunk):
            co = slice(128 * mi, 128 * mi + 128)
            Wf = [sign[co, 0:128, k].T for k in range(3)]
            Wx = [sign[co, 128:192, k].T for k in range(3)]  # [64, 128]
            sfx = str(mi) if i == 4 else ""
            comb = np.zeros((128, 128), np.float32)
            comb[0:64, :] = Wx[1]
            comb[64:128, :] = Wx[2]
            r = np.zeros((128, 128), np.float32)
            r[64:128, :] = Wx[0]
            bi = np.asarray(bs[i], np.float32) / sgs[i][1]
            put(f"l{i}c0_p{sfx}", pan(Wf[1], Wf[0]))
            put(f"l{i}c0_q{sfx}", pan(Wf[2], comb))
            put(f"l{i}c0_r{sfx}", pan(r, brow(bi[128 * mi:128 * mi + 128])))
        if cout == 192:
            Wc_ = [sign[128:192, 0:128, k].T for k in range(3)]  # [128,64]
            Wxc = [sign[128:192, 128:192, k].T for k in range(3)]  # [64,64]
            put(f"l{i}c1_a", pan(c1w(Wc_[1], None), c1w(Wc_[0], None)))
            put(f"l{i}c1_b", pan(c1w(None, Wc_[1]), c1w(Wc_[2], Wc_[0])))
            xc1 = quad(Wxc[1], None, Wxc[2], Wxc[0])
            put(f"l{i}c1_c", pan(c1w(None, Wc_[2]), xc1))
            xd0 = quad(None, None, Wxc[0], None)
            xd1 = quad(None, Wxc[1], None, Wxc[2])
            put(f"l{i}c1_d", pan(xd0, xd1))

    # --- bias + scale ---
    bcol = 0
    for i, (cin, cout, k, s, p) in enumerate(LAYERS):
        scale = sgs[i][1]
        b = np.asarray(bs[i], np.float32)
        bpk[0:128, bcol] = b[0:128]
        bpk[0:128, bcol + 1] = scale
        bcol += 2
        if cout > 128:
            if cout == 192:   # stacked for phase-stacked psum
                bpk[0:64, bcol] = b[128:192]
                bpk[64:128, bcol] = b[128:192]
            else:
                bpk[0:128, bcol] = b[128:256]
            bpk[0:128, bcol + 1] = scale
            bcol += 2
    return wpk16, wpk8.astype(E4NP), bpk


def _prep_x(xb):
    """Per-core L0 input, phase-reordered: cols [0:16000) hold the
    window for even L0 outputs, [16000:) for odd. xr[j, col] =
    xpad[5t + j] with t = 2*col (col<16000) or 2*(col-16000)+1."""
    xpad = np.zeros(T_IN + 16, np.float16)
    xpad[4:4 + T_IN] = xb.astype(np.float16)
    L = LOUT[0]
    xr = np.empty((11, L), np.float16)
    for j in range(10):
        xr[j, :] = xpad[j:j + 5 * L:5]
    xr[10, :] = 1.0    # bias-injection row
    return np.concatenate([xr[:, 0::2], xr[:, 1::2]], axis=1)


_CACHE = {}


def _build(act_name="Gelu", scales=(1.0,) * 7):
    """Build + compile the Bass program. The 7 ternary scales are baked
    in as ACT immediates (explicit scale/bias APs cost ~270ns per act)."""
    key = ("nc", act_name) + tuple(float(np.float32(s)) for s in scales)
    if key in _CACHE:
        return _CACHE[key]
    from concourse import bacc
    import concourse.mybir as mybir
    import concourse.tile as tile
    import concourse.bass as bass

    F8 = mybir.dt.float8e4
    F16 = mybir.dt.float16
    F32 = mybir.dt.float32
    ACTF = getattr(mybir.ActivationFunctionType, act_name)
    DRM = mybir.MatmulPerfMode.DoubleRow

    nc = bacc.Bacc("TRN2")
    xr_d = nc.dram_tensor("xr", [11, LOUT[0]], F16, kind="ExternalInput")
    on_d = nc.dram_tensor("on8", [128, PA[0]], F8, kind="ExternalInput")
    w16_d = nc.dram_tensor("w16", [128, W16TOT], F16, kind="ExternalInput")
    w8_d = nc.dram_tensor("w8", [128, W8TOT], F8, kind="ExternalInput")
    bp_d = nc.dram_tensor("bp", [128, 26], F32, kind="ExternalInput")
    y_d = nc.dram_tensor("y", [256, 500], F32, kind="ExternalOutput")

    with tile.TileContext(nc) as tc:
        pools = []

        def mkpool(name, bufs=1, space="SBUF"):
            p = tc.alloc_tile_pool(name=name, bufs=bufs, space=space)
            pools.append(p)
            return p

        wpool = mkpool("wpool")
        wt16 = wpool.tile([128, W16TOT], F16, name="wt16")
        wt8 = wpool.tile([128, W8TOT], F8, name="wt8")
        bt = wpool.tile([128, 26], F32, name="bt")

        opool = mkpool("opool")
        stage = opool.tile([128, 1000], F32, name="stage")
        scratch = opool.tile([128, 512], F16, name="scratch")
        xpool = mkpool("xpool", bufs=3)


        lpool = mkpool("lpool")
        A0 = lpool.tile([128, NREG[0], PA[0]], F8, name="A0")
        A1 = lpool.tile([128, NREG[1], PA[1]], F8, name="A1")
        A2 = lpool.tile([128, NREG[2], PA[2]], F8, name="A2")
        A3 = lpool.tile([128, NREG[3], PA[3]], F8, name="A3")
        A4a = lpool.tile([128, 2, 1004], F16, name="A4a")
        A4b = lpool.tile([128, 2, 1004], F16, name="A4b")
        A5a = lpool.tile([128, 2, 504], F16, name="A5a")
        A5b = lpool.tile([128, 2, 504], F16, name="A5b")
        AT = {0: A0, 1: A1, 2: A2, 3: A3}

        # zero halos (col 0 of every region; e-tail halo for L5/L6 srcs)
        for i in (0, 1, 2, 3):
            t, P, nr = AT[i], PA[i], NREG[i]
            for r in range(nr):
                nc.vector.memset(
                    bass.AP(t.tensor, r * P, [[nr * P, 128], [1, 1]]), 0.0)
        for i in (0, 1, 2, 3):    # ones region = last region index
            t, P, nr = AT[i], PA[i], NREG[i]
            nc.sync.dma_start(
                out=bass.AP(t.tensor, (nr - 1) * P, [[nr * P, 128], [1, P]]),
                in_=on_d.ap()[:, 0:P])
        for t in (A4a, A4b):
            nc.vector.memset(t[:, 1, 0:1], 0.0)       # o halo
            nc.vector.memset(t[:, 0, 1001:1002], 0.0)  # e[1000]
        for t in (A5a, A5b):
            nc.vector.memset(t[:, 1, 0:1], 0.0)
            nc.vector.memset(t[:, 0, 501:502], 0.0)

        # PSUM: one deep rotating pool - 8 units in flight in phase A
        # ([128,1024] x 8 banks), 4 in phase B ([128,2048] x 4).
        poolPS = tc.alloc_tile_pool(name="poolPS", bufs=4, space="PSUM")
        cur = {"pool": poolPS, "w": 1024}

        def ps_tile():
            return cur["pool"].tile([128, cur["w"]], F32, name="pps",
                                    tag="ps")

        fa_tile = ps_tile
        fb_tile = ps_tile

        nc.vector.memset(scratch[:, :], 0.0)

        def junk_mms(n):
            jp = fb_tile()
            for _ in range(n):
                nc.tensor.matmul(jp[:, 0:512], scratch[:, 0:128],
                                 scratch[:, :], start=True, stop=True)

        junk_mms(14)

        def w8ap(name, panels):
            c = W8COL[name]
            if panels == 2:
                return bass.AP(wt8.tensor, c, [[W8TOT, 128], [128, 2],
                                               [1, 128]])
            return bass.AP(wt8.tensor, c, [[W8TOT, 128], [1, 128]])

        def act_eo(dst, i, mi, u0, n, ps, pitchcols, P, with_bias=True):
            """One-call phase-scatter GELU: psum [128, n] -> e/o regions
            (fp16 dst tiles: direct from ACT)."""
            c = _bcol(i, mi)
            kw = dict(bias=bt[0:128, c:c + 1]) if with_bias else {}
            nc.scalar.activation(
                bass.AP(dst.tensor, 1 + u0 // 2,
                        [[pitchcols, 128], [P, 2], [1, n // 2]]),
                bass.AP(ps.tensor, ps.offset,
                        [[ps.ap[0][0], 128], [1, 2], [2, n // 2]]),
                ACTF, scale=float(scales[i]), **kw)

        def act_eo8(dst, i, mi, u0, n, ps, pitchcols, P):
            """Phase-scatter GELU, fp8 dst, bias pre-injected via matmul
            ones-panels, scale as immediate: minimal ACT overhead."""
            nc.scalar.activation(
                bass.AP(dst.tensor, 1 + u0 // 2,
                        [[pitchcols, 128], [P, 2], [1, n // 2]]),
                bass.AP(ps.tensor, ps.offset,
                        [[ps.ap[0][0], 128], [1, 2], [2, n // 2]]),
                ACTF, scale=float(scales[i]))

        def act_c18(dstap, ps, h, i, mi, with_bias):
            """c1 (stacked) GELU -> X region. deep-c1 has no free DR slot
            for bias injection, so those acts keep an explicit bias AP."""
            c = _bcol(i, mi)
            if with_bias:
                nc.scalar.activation(dstap, ps[0:128, 0:h], ACTF,
                                     bias=bt[0:128, c:c + 1],
                                     scale=float(scales[i]))
            else:
                nc.scalar.activation(dstap, ps[0:128, 0:h], ACTF,
                                     scale=float(scales[i]))

        def act_plain(dstap, srcap, i, mi, with_bias=True):
            c = _bcol(i, mi)
            if with_bias:
                nc.scalar.activation(dstap, srcap, ACTF,
                                     bias=bt[0:128, c:c + 1],
                                     scale=float(scales[i]))
            else:
                nc.scalar.activation(dstap, srcap, ACTF,
                                     scale=float(scales[i]))

        # ---------------- L1 units ----------------
        def l1_c0_unit(u0, n):
            ps = fa_tile()
            P = PA[0]
            for s0 in range(0, n, NT):
                w = min(NT, n - s0)
                u = u0 + s0
                nc.tensor.matmul(
                    ps[:, s0:s0 + w], w8ap("l1c0_p", 2),
                    bass.AP(A0.tensor, u + 1,
                            [[3 * P, 128], [P - 1, 2], [1, w]]),
                    start=True, stop=False, perf_mode=DRM)
                nc.tensor.matmul(
                    ps[:, s0:s0 + w], w8ap("l1c0_s", 2),
                    bass.AP(A0.tensor, P + u + 1,
                            [[3 * P, 128], [P, 2], [1, w]]),
                    start=False, stop=True, perf_mode=DRM)
            act_eo8(A1, 1, 0, u0, n, ps, 5 * PA[1], PA[1])

        def l1_c1_unit(m0, h):
            ps = fb_tile()
            P = PA[0]
            for b0 in range(0, h, NT):
                w = min(NT, h - b0)
                m = m0 + b0
                nc.tensor.matmul(
                    ps[:, b0:b0 + w], w8ap("l1c1_a", 2),
                    bass.AP(A0.tensor, 2 * m + 1,
                            [[3 * P, 128], [P - 1, 2], [2, w]]),
                    start=True, stop=False, perf_mode=DRM)
                nc.tensor.matmul(
                    ps[:, b0:b0 + w], w8ap("l1c1_b", 2),
                    bass.AP(A0.tensor, 2 * m + 2,
                            [[3 * P, 128], [P - 1, 2], [2, w]]),
                    start=False, stop=False, perf_mode=DRM)
                nc.tensor.matmul(
                    ps[:, b0:b0 + w], w8ap("l1c1_c", 2),
                    bass.AP(A0.tensor, P + 2 * m + 2,
                            [[3 * P, 128], [P - 1, 2], [2, w]]),
                    start=False, stop=True, perf_mode=DRM)
            P1 = PA[1]
            act_c18(bass.AP(A1.tensor, 2 * P1 + m0 + 1,
                            [[5 * P1, 128], [1, h]]), ps, h, 1, 1, False)
            nc.sync.dma_start(
                out=bass.AP(A1.tensor, 3 * P1 + m0 + 1,
                            [[5 * P1, 128], [1, h]]),
                in_=bass.AP(A1.tensor, 2 * P1 + m0 + 1,
                            [[5 * P1, 128], [1, h]]))

        # ---------------- deep units (L2-L4) ----------------
        def deep_c0_unit(i, u0, n, mi):
            src, P = AT[i - 1], PA[i - 1]
            nr = NREG[i - 1]
            sfx = str(mi) if i == 4 else ""
            ps = fa_tile()
            for s0 in range(0, n, NT):
                w = min(NT, n - s0)
                u = u0 + s0
                nc.tensor.matmul(
                    ps[:, s0:s0 + w], w8ap(f"l{i}c0_p{sfx}", 2),
                    bass.AP(src.tensor, u + 1,
                            [[nr * P, 128], [P - 1, 2], [1, w]]),
                    start=True, stop=False, perf_mode=DRM)
                nc.tensor.matmul(
                    ps[:, s0:s0 + w], w8ap(f"l{i}c0_q{sfx}", 2),
                    bass.AP(src.tensor, P + u + 1,
                            [[nr * P, 128], [P, 2], [1, w]]),
                    start=False, stop=False, perf_mode=DRM)
                # panels (X col u, ones col u): ones = last region
                nc.tensor.matmul(
                    ps[:, s0:s0 + w], w8ap(f"l{i}c0_r{sfx}", 2),
                    bass.AP(src.tensor, 2 * P + u,
                            [[nr * P, 128], [(nr - 3) * P, 2], [1, w]]),
                    start=False, stop=True, perf_mode=DRM)
            if i < 4:
                act_eo8(AT[i], i, 0, u0, n, ps, NREG[i] * PA[i], PA[i])
            else:
                dst = A4a if mi == 0 else A4b
                act_eo(dst, 4, mi, u0, n, ps, 2 * 1004, 1004, False)

        def deep_c1_unit(i, m0, h):
            src, P = AT[i - 1], PA[i - 1]
            nr = NREG[i - 1]
            ps = fb_tile()
            for b0 in range(0, h, NT):
                w = min(NT, h - b0)
                m = m0 + b0
                nc.tensor.matmul(
                    ps[:, b0:b0 + w], w8ap(f"l{i}c1_a", 2),
                    bass.AP(src.tensor, 2 * m + 1,
                            [[nr * P, 128], [P - 1, 2], [2, w]]),
                    start=True, stop=False, perf_mode=DRM)
                nc.tensor.matmul(
                    ps[:, b0:b0 + w], w8ap(f"l{i}c1_b", 2),
                    bass.AP(src.tensor, 2 * m + 2,
                            [[nr * P, 128], [P - 1, 2], [2, w]]),
                    start=False, stop=False, perf_mode=DRM)
                nc.tensor.matmul(
                    ps[:, b0:b0 + w], w8ap(f"l{i}c1_c", 2),
                    bass.AP(src.tensor, P + 2 * m + 2,
                            [[nr * P, 128], [P - 1, 2], [2, w]]),
                    start=False, stop=False, perf_mode=DRM)
                nc.tensor.matmul(
                    ps[:, b0:b0 + w], w8ap(f"l{i}c1_d", 2),
                    bass.AP(src.tensor, 2 * P + 2 * m,
                            [[nr * P, 128], [P + 2, 2], [2, w]]),
                    start=False, stop=True, perf_mode=DRM)
            Pi = PA[i]
            nri = NREG[i]
            act_c18(bass.AP(AT[i].tensor, 2 * Pi + m0 + 1,
                            [[nri * Pi, 128], [1, h]]), ps, h, i, 1, True)
            if nri == 5:    # tiles with an Xdup region (A1, A2)
                nc.sync.dma_start(
                    out=bass.AP(AT[i].tensor, 3 * Pi + m0 + 1,
                                [[nri * Pi, 128], [1, h]]),
                    in_=bass.AP(AT[i].tensor, 2 * Pi + m0 + 1,
                                [[nri * Pi, 128], [1, h]]))

        # ---------------- L5/L6 std units (fp16) ----------------
        def std_unit(i, mi):
            (ta, tb) = (A4a, A4b) if i == 5 else (A5a, A5b)
            Pp = 1004 if i == 5 else 504
            base = W16_L5 if i == 5 else W16_L6
            lout = LOUT[i]
            ps = ps_tile()
            g = 0
            for ti in range(2):
                t = ta if ti == 0 else tb
                # taps: o[v-1] (o col v), e[v] (e col v+1),
                #       o[v] (o col v+1), e[v+1] (e col v+2)
                srcs = [(Pp, 0), (0, 1), (Pp, 1), (0, 2)]
                for kk in range(4):
                    lhsT = wt16[0:128, base + g * 256 + 128 * mi:
                                base + g * 256 + 128 * mi + 128]
                    roff, cofs = srcs[kk]
                    for s0 in range(0, lout, NT):
                        w = min(NT, lout - s0)
                        nc.tensor.matmul(
                            ps[:, s0:s0 + w], lhsT,
                            bass.AP(t.tensor, roff + s0 + cofs,
                                    [[2 * Pp, 128], [1, w]]),
                            start=(g == 0), stop=(g == 7))
                    g += 1
            if i == 5:
                dst = A5a if mi == 0 else A5b
                act_eo(dst, 5, mi, 0, lout, ps, 2 * 504, 504)
            else:
                act_plain(stage[0:128, 500 * mi:500 * mi + lout],
                          ps[0:128, 0:lout], 6, mi)

        # ============ phase A: L0 streamed; L1 + early L2 woven ========
        wrest = [0]

        def after_first_xt():
            if wrest[0] == 1:
                nc.gpsimd.dma_start(out=wt8[:, :], in_=w8_d.ap())
                nc.gpsimd.dma_start(out=wt16[:, 128:W16TOT],
                                    in_=w16_d.ap()[:, 128:W16TOT])
            wrest[0] += 1

        n_ch = (LOUT[0] + A0C - 1) // A0C
        for c in range(n_ch):
            cb2 = c * A0H
            csz2 = min(A0H, LOUT[0] // 2 - cb2)
            fillers = []
            if c > 0:
                pb = (c - 1) * A0C // 2     # L1 position base of chunk c-1
                for j in range(4):
                    fillers.append(lambda u0=pb + 1024 * j:
                                   l1_c0_unit(u0, 1024))
                    if j % 2 == 1:
                        fillers.append(lambda m0=pb // 2 + 512 * (j - 1):
                                       l1_c1_unit(m0, 1024))
            if c >= 2:
                for j in range(2):
                    fillers.append(lambda u0=(c - 2) * 2048 + 1024 * j:
                                   deep_c0_unit(2, u0, 1024, 0))
                fillers.append(lambda m0=(c - 2) * 1024:
                               deep_c1_unit(2, m0, 1024))
            if c == 0:
                fillers = [lambda: junk_mms(2) for _ in range(4)]
            fi = 0
            nst = 2 * ((csz2 + 1023) // 1024)
            for sti in range(nst):
                ph = sti % 2
                t0 = cb2 + 1024 * (sti // 2)
                stw = min(1024, cb2 + csz2 - t0)
                xt = xpool.tile([128, NT], F16, tag="xt",
                                name=f"xt{ph}_{t0}")
                if t0 == 0 and ph == 0:
                    nc.sync.dma_start(out=wt16[:, 0:128],
                                      in_=w16_d.ap()[:, 0:128])
                    nc.sync.dma_start(out=bt[:, :], in_=bp_d.ap())
                for s in range(0, stw, NT):
                    w = min(NT, stw - s)
                    g = (s // NT) * 64
                    nc.sync.dma_start(
                        out=xt[g:g + 11, 0:w],
                        in_=xr_d.ap()[:, 16000 * ph + t0 + s:
                                      16000 * ph + t0 + s + w])
                after_first_xt()
                ps = ps_tile()
                for s in range(0, stw, NT):
                    w = min(NT, stw - s)
                    g = (s // NT) * 64
                    nc.tensor.matmul(ps[:, s:s + w],
                                     wt16[g:g + 11, 0:128],
                                     xt[g:g + 11, 0:w],
                                     start=True, stop=True,
                                     tile_position=(g, 0))
                act_plain(bass.AP(A0.tensor, ph * PA[0] + t0 + 1,
                                  [[3 * PA[0], 128], [1, stw]]),
                          ps[0:128, 0:stw], 0, 0, False)
                if sti % 2 == 1 and fi < len(fillers):
                    fillers[fi]()
                    fi += 1
            while fi < len(fillers):
                fillers[fi]()
                fi += 1
        # drain: L1 of last chunk (clamped to LOUT[1])
        pb = (n_ch - 1) * A0C // 2
        for j in range(4):
            u0 = pb + 1024 * j
            if u0 < LOUT[1]:
                l1_c0_unit(u0, min(1024, LOUT[1] - u0))
            if j % 2 == 1:
                m0 = pb // 2 + 512 * (j - 1)
                if m0 < LOUT[1] // 2:
                    l1_c1_unit(m0, min(1024, LOUT[1] // 2 - m0))

        # ============ phase B: rest of L2, then L3..L6 =================

        deep_c0_unit(2, 4096, 1024, 0)
        deep_c1_unit(2, 2048, 1024)
        deep_c0_unit(2, 5120, 1024, 0)
        deep_c1_unit(2, 3072, 928)
        deep_c0_unit(2, 6144, 1024, 0)
        deep_c0_unit(2, 7168, 832, 0)
        deep_c0_unit(3, 0, 1024, 0)
        deep_c1_unit(3, 0, 1024)
        deep_c0_unit(3, 1024, 1024, 0)
        deep_c1_unit(3, 1024, 976)
        deep_c0_unit(3, 2048, 1024, 0)
        deep_c0_unit(3, 3072, 928, 0)
        deep_c0_unit(4, 0, 1024, 0)
        deep_c0_unit(4, 0, 1024, 1)
        deep_c0_unit(4, 1024, 976, 0)
        deep_c0_unit(4, 1024, 976, 1)
        std_unit(5, 0)
        std_unit(5, 1)
        std_unit(6, 0)
        std_unit(6, 1)

        nc.sync.dma_start(out=y_d.ap()[0:128, :], in_=stage[:, 0:500])
        nc.sync.dma_start(out=y_d.ap()[128:256, :], in_=stage[:, 500:1000])
        poolPS.release()
        for p in reversed(pools):
            p.release()

    nc.compile()
    _CACHE[key] = nc
    return nc


def kernel(x, w0, b0, w1, b1, w2, b2, w3, b3, w4, b4, w5, b5, w6, b6):
    import os
    from concourse.bass_utils import run_bass_kernel_spmd

    ws = [w0, w1, w2, w3, w4, w5, w6]
    bs = [b0, b1, b2, b3, b4, b5, b6]
    wpk16, wpk8, bpk = _pack_host(ws, bs)
    scales = tuple(_signs(w)[1] for w in ws)
    ones8 = np.ones((128, PA[0]), E4NP)
    x = np.asarray(x, np.float32)
    in_maps = [{"xr": _prep_x(x[b]), "w16": wpk16, "w8": wpk8, "bp": bpk,
                "on8": ones8}
               for b in range(N_CORES)]
    nc = _build("Gelu", scales)
    trace = bool(os.environ.get("BITCONV_TRACE"))
    res = run_bass_kernel_spmd(nc, in_maps, core_ids=list(range(N_CORES)),
                               trace=trace)
    if trace:
        print(f"HW exec time: {res.exec_time_ns} ns")
        _CACHE["last_results"] = res
    return np.stack([res.results[b]["y"] for b in range(N_CORES)], axis=0)
